# revision 57
# baseline (speedup 1.0000x reference)
"""BiAttention kernel for Trainium2, 8 NeuronCores, data-parallel over batch.

Reference computation (per batch b):
    S[i,j] = w1.c_i + w2.q_j + w3.(c_i*q_j)
    A      = softmax(S, axis=j)
    U[i]   = sum_j A[i,j] q_j
    bmax_i = max_j A[i,j]
    h      = sum_i bmax_i c_i
    G      = concat([c, U, c*U, c*h], axis=-1)

Structure (j-major, bf16 matmul operands):
  - softmax over j is invariant to the s_c[i] term -> w1 is dead.
  - S^T[j,i] is computed directly (lhsT = w3-scaled q^T, rhs = c^T, both
    bf16: 1 cyc/col streaming, FWL-fast weight loads, 1 cyc/row PE
    transposes).  Then:
      * s_q[j] is the per-partition BIAS of the exp activation (free).
      * exp(S^T) IS A^T, exactly the lhsT layout the U matmul needs.
      * Z_i falls out of the U matmul via a ones-column appended to q.
      * bmax_i via a 3-op bf16 tensor_max tree (DVE 2x mode) + PE
        transposes + X-axis max reduce.
  - The i axis is processed in superblocks [1024,1024,1024,512,512]; the
    U matmul for a block runs during the next block's S phase; the two
    trailing 512-wide blocks shrink the un-overlapped U tail.
  - h = sum_i b_i c_i via PE (bf16), h broadcast via a K=1 ones matmul.
"""

import sys

if "/opt/trn_rl_repo" not in sys.path:
    sys.path.insert(0, "/opt/trn_rl_repo")

from contextlib import ExitStack

import numpy as np

import concourse.bass as bass
import concourse.bacc as bacc_mod
import concourse.tile as tile
from concourse import mybir
from concourse.bass_utils import run_bass_kernel_spmd
from concourse.masks import make_identity

B, Tc, Tq, D = 8, 4096, 1024, 256
P = 128
NT = Tc // P  # 32 context row-tiles
JC = Tq // P  # 8 question j-tiles
KC = D // P  # 2 feature chunks
N_CORES = 8
BLOCKS = [(0, 1024), (1024, 1024), (2048, 1024), (3072, 512), (3584, 512)]
F32 = mybir.dt.float32
BF16 = mybir.dt.bfloat16
EXP = mybir.ActivationFunctionType.Exp
MAX = mybir.AluOpType.max
AXX = mybir.AxisListType.X


def _build_program() -> bass.Bass:
    nc = bacc_mod.Bacc()
    c_dram = nc.declare_dram_parameter("context", [Tc, D], F32, isOutput=False)
    q_dram = nc.declare_dram_parameter("question", [Tq, D], F32, isOutput=False)
    w_dram = nc.declare_dram_parameter("w", [3 * D, 1], F32, isOutput=False)
    g_dram = nc.declare_dram_parameter("out", [Tc, 4 * D], F32, isOutput=True)

    with ExitStack() as ctx:
        tc = ctx.enter_context(tile.TileContext(nc))
        singles = ctx.enter_context(tc.tile_pool(name="sb", bufs=1))
        big2 = singles
        work = singles
        psp = ctx.enter_context(tc.tile_pool(name="ps", bufs=2, space="PSUM"))
        ps_s = psp
        ps_tp = psp
        ps_u = psp

        st = {"nc": nc, "big2": big2, "ps_u": ps_u}

        # ---------------- prep ----------------
        # question: raw fp32 load first, in halves (it heads the critical path)
        q_raw = big2.tile([P, JC, D], F32, tag="qu", bufs=2)
        for jh in range(2):
            nc.sync.dma_start(
                out=q_raw[:, jh * 4 : (jh + 1) * 4, :],
                in_=q_dram[jh * 512 : (jh + 1) * 512, :].rearrange(
                    "(jc p) d -> p jc d", p=P
                ),
            )

        ident = singles.tile([P, P], F32)
        make_identity(nc, ident)
        identb = singles.tile([P, P], BF16)
        nc.vector.tensor_copy(identb, ident)

        # w2|w3 in one DMA on the ACT ring: rows 256..767 as [128, 4]
        wtmp = singles.tile([P, 2 * KC], F32)
        nc.scalar.dma_start(
            out=wtmp, in_=w_dram[D : 3 * D, 0:1].rearrange("(k p) o -> p (k o)", p=P)
        )
        w3sc = wtmp[:, KC : 2 * KC]
        w2sc = singles.tile([P, KC], BF16)
        nc.vector.tensor_copy(w2sc, wtmp[:, 0:KC])

        # bf16 copy of q with a ones column, per half as the halves land
        q_aug = singles.tile([P, JC, D + 8], BF16)
        nc.vector.memset(q_aug[:, :, D : D + 8], 0.0)
        nc.vector.memset(q_aug[:, :, D : D + 1], 1.0)
        for jh in range(2):
            nc.vector.tensor_copy(
                q_aug[:, jh * 4 : (jh + 1) * 4, 0:D], q_raw[:, jh * 4 : (jh + 1) * 4, :]
            )

        # q^T via PE transposes of the bf16 q; evacuated twice:
        # w3-scaled (S^T lhsT) and raw (s_q matvec rhs)
        qw3T = singles.tile([P, KC, Tq], BF16)
        qTr = singles.tile([P, KC, Tq], BF16)
        for jg in range(2):
            for kc in range(KC):
                tp = ps_tp.tile([P, 512], BF16, tag="tp")
                for j4 in range(4):
                    jc = jg * 4 + j4
                    nc.tensor.transpose(
                        tp[:, j4 * P : (j4 + 1) * P],
                        q_aug[:, jc, kc * P : (kc + 1) * P],
                        identb,
                    )
                nc.scalar.copy(qTr[:, kc, jg * 512 : (jg + 1) * 512], tp)
                nc.vector.tensor_scalar_mul(
                    qw3T[:, kc, jg * 512 : (jg + 1) * 512], tp, w3sc[:, kc : kc + 1]
                )

        # context loads in 512KB chunks; bf16 casts + c output segment writes
        c_all = singles.tile([P, NT, D], F32)
        c_b16 = singles.tile([P, NT, D], BF16)
        for cg in range(8):
            t0 = cg * 4
            nc.sync.dma_start(
                out=c_all[:, t0 : t0 + 4, :],
                in_=c_dram[t0 * P : (t0 + 4) * P, :].rearrange("(g p) d -> p g d", p=P),
            )
            if cg % 2 == 0:
                nc.scalar.copy(c_b16[:, t0 : t0 + 4, :], c_all[:, t0 : t0 + 4, :])
            else:
                nc.vector.tensor_copy(
                    c_b16[:, t0 : t0 + 4, :], c_all[:, t0 : t0 + 4, :]
                )
            nc.gpsimd.dma_start(
                out=g_dram[t0 * P : (t0 + 4) * P, 0:D].rearrange(
                    "(g p) d -> p g d", p=P
                ),
                in_=c_all[:, t0 : t0 + 4, :],
            )

        def emit_ct_chunk(cg):
            # c^T (bf16) for one 512-column chunk via PE transposes
            for th in range(2):
                tp2 = ps_tp.tile([P, 512], BF16, tag="tp", name=f"ctp_{cg}_{th}")
                tb = cg * 4 + th * 2
                for kc in range(KC):
                    for i2 in range(2):
                        nc.tensor.transpose(
                            tp2[:, kc * 256 + i2 * P : kc * 256 + (i2 + 1) * P],
                            c_b16[:, tb + i2, kc * P : (kc + 1) * P],
                            identb,
                        )
                for kc in range(KC):
                    dst = cT[:, kc, tb * P : (tb + 2) * P]
                    src = tp2[:, kc * 256 : (kc + 1) * 256]
                    if (th + kc) % 2 == 0:
                        nc.vector.tensor_copy(dst, src)
                    else:
                        nc.scalar.copy(dst, src)

        # s_q = q @ w2 as [1, Tq], then moved to per-partition [P, JC] via
        # K=1 matmuls against a ones [1,1] rhs
        sq_row = singles.tile([1, Tq], F32)
        for nb in range(2):
            sq_ps = ps_u.tile([1, 512], F32, tag="u")
            for kc in range(KC):
                nc.tensor.matmul(
                    sq_ps,
                    lhsT=w2sc[:, kc : kc + 1],
                    rhs=qTr[:, kc, nb * 512 : (nb + 1) * 512],
                    start=(kc == 0),
                    stop=(kc == KC - 1),
                )
            nc.vector.tensor_copy(sq_row[:, nb * 512 : (nb + 1) * 512], sq_ps)
        ones11 = singles.tile([1, 1], F32)
        nc.vector.memset(ones11, 1.0)
        sqT = singles.tile([P, JC], F32)
        tpq = ps_tp.tile([P, JC], F32, tag="tp")
        for jc in range(JC):
            nc.tensor.matmul(
                tpq[:, jc : jc + 1],
                lhsT=sq_row[:, jc * P : (jc + 1) * P],
                rhs=ones11,
                start=True,
                stop=True,
            )
        nc.vector.tensor_copy(sqT, tpq)

        # c^T chunks for block 0 now; the rest just-in-time in the main loop
        cT = singles.tile([P, KC, Tc], BF16)
        for cg in range(2):
            emit_ct_chunk(cg)

        # persistent accumulators
        b_raw = singles.tile([P, NT], F32)
        rZ_all = singles.tile([P, NT], F32)
        b_allb = singles.tile([P, NT], BF16)
        st.update(q_aug=q_aug, c_all=c_all, rZ_all=rZ_all, b_raw=b_raw)

        # ---------------- main loop over i-superblocks ----------------
        prev = None  # (block-index, A_sb)
        pending_fin = None
        for bi, (i0, W) in enumerate(BLOCKS):
            A_sb = work.tile([P, JC, W], BF16, tag="A", bufs=2, name=f"A_{bi}")
            if bi + 1 < len(BLOCKS):
                ni0, nW = BLOCKS[bi + 1]
                for cg in range(ni0 // 512, (ni0 + nW) // 512):
                    emit_ct_chunk(cg)
            # distribute the previous block's U i-tiles over this block's
            # 8 jt steps
            usched = [[] for _ in range(JC)]
            if prev is not None:
                tpw_prev = BLOCKS[prev[0]][1] // P
                for k in range(tpw_prev):
                    usched[k * JC // tpw_prev].append(k)
            for jt in range(JC):
                s_ps = ps_s.tile([P, W], F32, tag="s")
                for nh in range(W // 512):
                    sl = slice(nh * 512, (nh + 1) * 512)
                    isl = slice(i0 + nh * 512, i0 + (nh + 1) * 512)
                    for kc in range(KC):
                        nc.tensor.matmul(
                            s_ps[:, sl],
                            lhsT=qw3T[:, kc, jt * P : (jt + 1) * P],
                            rhs=cT[:, kc, isl],
                            start=(kc == 0),
                            stop=(kc == KC - 1),
                        )
                nc.scalar.activation(
                    A_sb[:, jt, :], s_ps, EXP, bias=sqT[:, jt : jt + 1]
                )
                if prev is not None:
                    for isub in usched[jt]:
                        _u_step(st, prev[0], isub, prev[1])
                if jt == 3 and pending_fin is not None:
                    # previous block's bmax finalize: by now its DVE max
                    # tree is long done, and the PE has S-matmuls queued
                    # ahead, so these transposes don't stall the PE.
                    _finalize_bmax(st, ps_tp, identb, *pending_fin)
                    pending_fin = None
            if prev is not None:
                _finish_block(st, prev[0], g_dram)
            if prev is not None and prev[0] == 2:
                # b for blocks 0-2 (i-tiles 0..23) is final: start h here
                nc.vector.tensor_copy(b_allb[:, 0:24], b_raw[:, 0:24])
                h_ps1 = ps_u.tile([1, D], F32, tag="u")
                for t in range(24):
                    nc.tensor.matmul(
                        h_ps1,
                        lhsT=b_allb[:, t : t + 1],
                        rhs=c_b16[:, t, :],
                        start=(t == 0),
                        stop=(t == 23),
                    )
                h_sb1 = singles.tile([1, D], F32)
                nc.vector.tensor_copy(h_sb1, h_ps1)

            # bmax over j: bf16 max tree on DVE, chunked into ~1us ops so
            # the strict-FIFO DVE queue never blocks the next block's U
            # evacuations behind a long op.  PE-transpose finalize deferred.
            bm4 = work.tile([P, 4, W], BF16, tag="bm4", bufs=1, name=f"bm4_{bi}")
            bm2 = work.tile([P, 2, W], BF16, tag="bm2", bufs=1, name=f"bm2_{bi}")
            bm1 = work.tile([P, W], BF16, tag="bm1", bufs=2, name=f"bm1_{bi}")
            CH = 256
            for c0 in range(0, W, CH):
                cs = slice(c0, c0 + CH)
                nc.vector.tensor_max(
                    bm4[:, :, cs], A_sb[:, 0:4, cs], A_sb[:, 4:8, cs]
                )
            for c0 in range(0, W, CH * 2):
                cs = slice(c0, c0 + CH * 2)
                nc.vector.tensor_max(bm2[:, :, cs], bm4[:, 0:2, cs], bm4[:, 2:4, cs])
            for c0 in range(0, W, CH * 2):
                cs = slice(c0, c0 + CH * 2)
                nc.vector.tensor_max(bm1[:, cs], bm2[:, 0, cs], bm2[:, 1, cs])
            pending_fin = (bm1, i0, W)
            prev = (bi, A_sb)

        for isub in range(BLOCKS[prev[0]][1] // P):
            _u_step(st, prev[0], isub, prev[1])
            if isub == 1:
                _finalize_bmax(st, ps_tp, identb, *pending_fin)
        _finish_block(st, prev[0], g_dram)

        # ---------------- epilogue: finish h (blocks 3-4), then c*h ----------------
        nc.vector.tensor_copy(b_allb[:, 24:NT], b_raw[:, 24:NT])
        h_ps2 = ps_u.tile([1, D], F32, tag="u")
        for t in range(24, NT):
            nc.tensor.matmul(
                h_ps2,
                lhsT=b_allb[:, t : t + 1],
                rhs=c_b16[:, t, :],
                start=(t == 24),
                stop=(t == NT - 1),
            )
        h_sb = singles.tile([1, D], BF16)
        nc.vector.tensor_add(h_sb, h_sb1, h_ps2)
        # broadcast h to all partitions with a K=1 ones matmul (bf16)
        ones1p = singles.tile([1, P], BF16)
        nc.vector.memset(ones1p, 1.0)
        hb_ps = ps_tp.tile([P, D], F32, tag="tp")
        nc.tensor.matmul(hb_ps, lhsT=ones1p, rhs=h_sb, start=True, stop=True)
        h_b = singles.tile([P, D], F32)
        nc.scalar.copy(h_b, hb_ps)
        for tg in range(8):
            ch4 = big2.tile([P, 4, D], F32, tag="ch", bufs=4, name=f"ch_{tg}")
            nc.vector.tensor_mul(
                ch4,
                c_all[:, tg * 4 : (tg + 1) * 4, :],
                h_b.unsqueeze(1).broadcast_to([P, 4, D]),
            )
            deng = nc.scalar if tg % 2 == 0 else nc.sync
            deng.dma_start(
                out=g_dram[tg * 4 * P : (tg + 1) * 4 * P, 3 * D : 4 * D].rearrange(
                    "(g p) d -> p g d", p=P
                ),
                in_=ch4,
            )

    nc.finalize()
    return nc


_UBLK = {}


def _finalize_bmax(st, ps_tp, identb, bm1, i0, W):
    """Cross-partition max of the tree result: PE transposes + X-reduce."""
    nc = st["nc"]
    for g in range(W // 512):
        tpb = ps_tp.tile([P, 4, P], BF16, tag="tp", name=f"tpb_{i0}_{g}")
        for k in range(4):
            nc.tensor.transpose(
                tpb[:, k, :], bm1[:, (g * 4 + k) * P : (g * 4 + k + 1) * P], identb
            )
        t0 = i0 // P + g * 4
        nc.vector.tensor_reduce(
            out=st["b_raw"][:, t0 : t0 + 4], in_=tpb, axis=AXX, op=MAX
        )


def _u_step(st, bi, isub, A_prev):
    """One i-tile of the U matmul for block bi, plus its U = Uraw/Z
    evacuation (alternating ACT / DVE)."""
    nc = st["nc"]
    i0, W = BLOCKS[bi]
    t = i0 // P + isub
    if isub == 0:
        tpw = W // P
        u_all = st["big2"].tile([P, tpw, D], F32, tag="qu", bufs=2, name=f"u_{bi}")
        cu_all = st["big2"].tile([P, tpw, D], F32, tag="qcu", bufs=2, name=f"cu_{bi}")
        _UBLK[bi % 2] = (u_all, cu_all)
    u_all, cu_all = _UBLK[bi % 2]
    u_ps = st["ps_u"].tile([P, D + 4], F32, tag="u", name=f"ups_{t}")
    for jt in range(JC):
        nc.tensor.matmul(
            u_ps,
            lhsT=A_prev[:, jt, isub * P : (isub + 1) * P],
            rhs=st["q_aug"][:, jt, 0 : D + 4],
            start=(jt == 0),
            stop=(jt == JC - 1),
        )
    rZ = st["rZ_all"][:, t : t + 1]
    nc.vector.reciprocal(rZ, u_ps[:, D : D + 1])
    if isub % 2 == 0:
        nc.scalar.mul(u_all[:, isub, :], u_ps[:, 0:D], rZ)
    else:
        nc.vector.tensor_scalar_mul(u_all[:, isub, :], u_ps[:, 0:D], rZ)


def _finish_block(st, bi, g_dram):
    """cU = c*U for the whole block (one DVE op), DMA out the U and c*U
    segments, and finish b = bmax/Z for its i-tiles."""
    nc = st["nc"]
    i0, W = BLOCKS[bi]
    tpw = W // P
    u_all, cu_all = _UBLK[bi % 2]
    t0 = i0 // P
    nc.vector.tensor_mul(cu_all, st["c_all"][:, t0 : t0 + tpw, :], u_all)
    nc.sync.dma_start(
        out=g_dram[i0 : i0 + W, D : 2 * D].rearrange("(g p) d -> p g d", p=P),
        in_=u_all,
    )
    nc.sync.dma_start(
        out=g_dram[i0 : i0 + W, 2 * D : 3 * D].rearrange("(g p) d -> p g d", p=P),
        in_=cu_all,
    )
    nc.vector.tensor_mul(
        st["b_raw"][:, t0 : t0 + tpw],
        st["b_raw"][:, t0 : t0 + tpw],
        st["rZ_all"][:, t0 : t0 + tpw],
    )


_NC_CACHE = None


def kernel(context, question, w):
    global _NC_CACHE
    context = np.asarray(context, dtype=np.float32)
    question = np.asarray(question, dtype=np.float32)
    w = np.asarray(w, dtype=np.float32)

    if _NC_CACHE is None:
        _NC_CACHE = _build_program()
    nc = _NC_CACHE

    in_maps = [
        {"context": context[b], "question": question[b], "w": w} for b in range(B)
    ]
    res = run_bass_kernel_spmd(nc, in_maps, list(range(N_CORES)))
    return np.stack([res.results[b]["out"] for b in range(B)], axis=0)
